# revision 5
# baseline (speedup 1.0000x reference)
"""Batched attention-score kernel for Trainium2 (Bass/Tile).

Computes scores = einsum("bsd,bd->bs", encoder_outputs, decoder_hidden)
for bsz=64, seq=2048, d_hid=1024 (fp32), returning [64, 1, 2048].

Strategy: data-parallel over 8 NeuronCores (8 batches per core). Each core
streams its 64 MiB shard of encoder_outputs through SBUF in large contiguous
DMAs (s-rows packed so each partition holds contiguous DRAM runs) and uses
the DVE fused tensor_tensor_reduce (out = in0*in1, accum_out = sum) against
a partition-broadcast copy of decoder_hidden. The kernel is HBM-bandwidth
bound: ~64 MiB / ~358 GB/s ~= 187 us per core.
"""

import sys

import numpy as np

sys.path.insert(0, "/opt/trn_rl_repo")

B, S, D = 64, 2048, 1024
NCORES = 8
BPC = B // NCORES  # batches per core
P = 128  # SBUF partitions

_NC_CACHE = {}


def build_nc(bpc=BPC, s=S, d=D, x=8, bufs=3):
    """Build the single-core Bass module.

    x = s-rows packed per partition per chunk. One chunk tile is
    [128, x*d] and covers 128*x consecutive s rows; per-partition DRAM
    reads are x*d*4 bytes contiguous.
    """
    from concourse import bacc, mybir, tile

    assert s % (P * x) == 0
    chunks = s // (P * x)

    nc = bacc.Bacc("TRN2", target_bir_lowering=False, debug=False)
    enc = nc.declare_dram_parameter("enc", [bpc, s, d], mybir.dt.float32, isOutput=False)
    dh = nc.declare_dram_parameter("dh", [bpc, d], mybir.dt.float32, isOutput=False)
    out = nc.declare_dram_parameter("out", [bpc, s], mybir.dt.float32, isOutput=True)

    with tile.TileContext(nc) as tc:
        with (
            tc.tile_pool(name="encp", bufs=bufs) as encp,
            tc.tile_pool(name="dhp", bufs=1) as dhp,
            tc.tile_pool(name="scp", bufs=2) as scp,
            tc.tile_pool(name="dummyp", bufs=2) as dummyp,
        ):
            # Broadcast all bpc decoder vectors across the 128 partitions once.
            dh_all = dhp.tile([P, bpc * d], mybir.dt.float32)
            nc.sync.dma_start(
                out=dh_all[:, :],
                in_=dh[:, :].rearrange("a b -> (a b)")[None, :].broadcast_to([P, bpc * d]),
            )

            for b in range(bpc):
                enc_b = enc[b].rearrange("(h p x) d -> h p (x d)", p=P, x=x)
                out_b = out[b].rearrange("(h p x) -> h p x", p=P, x=x)
                for h in range(chunks):
                    t = encp.tile([P, x * d], mybir.dt.float32, tag="enc")
                    nc.sync.dma_start(out=t[:, :], in_=enc_b[h])
                    sc = scp.tile([P, x], mybir.dt.float32, tag="sc")
                    dummy = dummyp.tile([P, 1], mybir.dt.float32, tag="dummy")
                    for j in range(x):
                        # out = (in0 * 1.0) * in1; accum_out = sum(out) along
                        # the free dim -> the s-row dot product, one DVE pass.
                        nc.vector.scalar_tensor_tensor(
                            out=dummy.broadcast_to([P, d]),
                            in0=t[:, j * d : (j + 1) * d],
                            scalar=1.0,
                            in1=dh_all[:, b * d : (b + 1) * d],
                            op0=mybir.AluOpType.mult,
                            op1=mybir.AluOpType.mult,
                            accum_out=sc[:, j : j + 1],
                        )
                    nc.sync.dma_start(out=out_b[h], in_=sc[:, :])
    nc.compile()
    return nc


def _get_nc():
    if "nc" not in _NC_CACHE:
        _NC_CACHE["nc"] = build_nc()
    return _NC_CACHE["nc"]


def run(decoder_hidden, encoder_outputs, trace=False, **run_kwargs):
    """Shard inputs over the 8 cores, run, gather. Returns (scores, results)."""
    from concourse.bass_utils import run_bass_kernel_spmd

    decoder_hidden = np.asarray(decoder_hidden, dtype=np.float32)
    encoder_outputs = np.asarray(encoder_outputs, dtype=np.float32)
    assert decoder_hidden.shape == (B, D)
    assert encoder_outputs.shape == (B, S, D)

    nc = _get_nc()
    in_maps = []
    for c in range(NCORES):
        sl = slice(c * BPC, (c + 1) * BPC)
        in_maps.append(
            {
                "enc": np.ascontiguousarray(encoder_outputs[sl]),
                "dh": np.ascontiguousarray(decoder_hidden[sl]),
            }
        )
    res = run_bass_kernel_spmd(nc, in_maps, list(range(NCORES)), trace=trace, **run_kwargs)
    scores = np.concatenate([res.results[c]["out"] for c in range(NCORES)], axis=0)
    return scores.reshape(B, 1, S), res


def kernel(decoder_hidden, encoder_outputs):
    return run(decoder_hidden, encoder_outputs)[0]


# revision 6
# speedup vs baseline: 1.0431x; 1.0431x over previous
"""Batched attention-score kernel for Trainium2 (Bass/Tile).

Computes scores = einsum("bsd,bd->bs", encoder_outputs, decoder_hidden)
for bsz=64, seq=2048, d_hid=1024 (fp32), returning [64, 1, 2048].

Strategy: data-parallel over 8 NeuronCores (8 batches per core). Each core
streams its 64 MiB shard of encoder_outputs through SBUF in large contiguous
DMAs (s-rows packed so each partition holds contiguous DRAM runs) and uses
the DVE fused tensor_tensor_reduce (out = in0*in1, accum_out = sum) against
a partition-broadcast copy of decoder_hidden. The kernel is HBM-bandwidth
bound: ~64 MiB / ~358 GB/s ~= 187 us per core.
"""

import sys

import numpy as np

sys.path.insert(0, "/opt/trn_rl_repo")

B, S, D = 64, 2048, 1024
NCORES = 8
BPC = B // NCORES  # batches per core
P = 128  # SBUF partitions

_NC_CACHE = {}


def build_nc(bpc=BPC, s=S, d=D, x=8, bufs=3):
    """Build the single-core Bass module.

    x = s-rows packed per partition per chunk. One chunk tile is
    [128, x*d] and covers 128*x consecutive s rows; per-partition DRAM
    reads are x*d*4 bytes contiguous.
    """
    from concourse import bacc, mybir, tile

    assert s % (P * x) == 0
    chunks = s // (P * x)

    nc = bacc.Bacc("TRN2", target_bir_lowering=False, debug=False)
    enc = nc.declare_dram_parameter("enc", [bpc, s, d], mybir.dt.float32, isOutput=False)
    dh = nc.declare_dram_parameter("dh", [bpc, d], mybir.dt.float32, isOutput=False)
    out = nc.declare_dram_parameter("out", [bpc, s], mybir.dt.float32, isOutput=True)

    with tile.TileContext(nc) as tc:
        with (
            tc.tile_pool(name="encp", bufs=bufs) as encp,
            tc.tile_pool(name="dhp", bufs=1) as dhp,
            tc.tile_pool(name="scp", bufs=2) as scp,
            tc.tile_pool(name="dummyp", bufs=2) as dummyp,
        ):
            # Load the bpc decoder vectors into partition 0, then replicate
            # across partitions on GPSIMD (keeps the SDMA rings free for the
            # encoder stream).
            dh_row = dhp.tile([1, bpc * d], mybir.dt.float32)
            nc.sync.dma_start(
                out=dh_row[:, :], in_=dh[:, :].rearrange("a b -> (a b)")[None, :]
            )
            dh_all = dhp.tile([P, bpc * d], mybir.dt.float32)
            for b in range(bpc):
                nc.gpsimd.partition_broadcast(
                    dh_all[:, b * d : (b + 1) * d], dh_row[0:1, b * d : (b + 1) * d]
                )

            # Alternate large encoder DMAs across both HWDGE rings (SP +
            # ACT) so one ring's fixed completion cost hides under the
            # other's data phase.
            rings = [nc.sync, nc.scalar]
            n_dma = 0
            for b in range(bpc):
                enc_b = enc[b].rearrange("(h p x) d -> h p (x d)", p=P, x=x)
                out_b = out[b].rearrange("(h p x) -> h p x", p=P, x=x)
                for h in range(chunks):
                    t = encp.tile([P, x * d], mybir.dt.float32, tag="enc")
                    if b == 0 and h == 0:
                        # Split the very first chunk so compute starts ~4x
                        # sooner (pipeline ramp).
                        q = (x * d) // 4
                        for k in range(4):
                            rings[k % 2].dma_start(
                                out=t[:, k * q : (k + 1) * q],
                                in_=enc_b[h][:, k * q : (k + 1) * q],
                            )
                    else:
                        rings[n_dma % 2].dma_start(out=t[:, :], in_=enc_b[h])
                    n_dma += 1
                    sc = scp.tile([P, x], mybir.dt.float32, tag="sc")
                    dummy = dummyp.tile([P, 1], mybir.dt.float32, tag="dummy")
                    for j in range(x):
                        # out = (in0 * 1.0) * in1; accum_out = sum(out) along
                        # the free dim -> the s-row dot product, one DVE pass.
                        nc.vector.scalar_tensor_tensor(
                            out=dummy.broadcast_to([P, d]),
                            in0=t[:, j * d : (j + 1) * d],
                            scalar=1.0,
                            in1=dh_all[:, b * d : (b + 1) * d],
                            op0=mybir.AluOpType.mult,
                            op1=mybir.AluOpType.mult,
                            accum_out=sc[:, j : j + 1],
                        )
                    # Tiny result stores go out via SWDGE (GPSIMD) to stay
                    # off the HWDGE rings feeding the encoder stream.
                    nc.gpsimd.dma_start(out=out_b[h], in_=sc[:, :])
    nc.compile()
    return nc


def _get_nc():
    if "nc" not in _NC_CACHE:
        _NC_CACHE["nc"] = build_nc()
    return _NC_CACHE["nc"]


def run(decoder_hidden, encoder_outputs, trace=False, **run_kwargs):
    """Shard inputs over the 8 cores, run, gather. Returns (scores, results)."""
    from concourse.bass_utils import run_bass_kernel_spmd

    decoder_hidden = np.asarray(decoder_hidden, dtype=np.float32)
    encoder_outputs = np.asarray(encoder_outputs, dtype=np.float32)
    assert decoder_hidden.shape == (B, D)
    assert encoder_outputs.shape == (B, S, D)

    nc = _get_nc()
    in_maps = []
    for c in range(NCORES):
        sl = slice(c * BPC, (c + 1) * BPC)
        in_maps.append(
            {
                "enc": np.ascontiguousarray(encoder_outputs[sl]),
                "dh": np.ascontiguousarray(decoder_hidden[sl]),
            }
        )
    res = run_bass_kernel_spmd(nc, in_maps, list(range(NCORES)), trace=trace, **run_kwargs)
    scores = np.concatenate([res.results[c]["out"] for c in range(NCORES)], axis=0)
    return scores.reshape(B, 1, S), res


def kernel(decoder_hidden, encoder_outputs):
    return run(decoder_hidden, encoder_outputs)[0]
